# revision 13
# baseline (speedup 1.0000x reference)
"""Trainium2 Bass kernel for nn_PolyAttn (B=4, N=2048, D=H=1024).

Mathematical structure exploited: the reference computes attention weights
a = (alpha*q@k^T + 1)^4 followed by a = a / |a|.  Since s^4 >= 0, the
normalized score matrix is exactly the all-ones matrix (independent of
alpha), so

    o[b, n, :] = (sum_m x[b, m, :]) @ W_v @ w_o        for every n,

where W_v = w_qkv[:, 2H:3H].  All arithmetic stays in fp32.  Two SPMD
launches over the 8 cores:

  Launch 1: each core owns a 1024-row slice of x (flattened [8192, 1024]).
     A running DVE add folds the 8 [128, 1024] tiles as they stream in;
     the remaining 128 partitions are folded by PE-transposing each
     [128, 128] column chunk into packed PSUM banks and free-dim-reducing
     them on DVE (two batched reduces).  Dummy PE matmuls issued against
     already-arrived tiles keep the PE activity monitor (HAM) warm so the
     transposes run at the fast clock.  Output is the per-core partial
     row-sum, transposed, as [128, 8].
     Host sums pairs of partials (the cross-core reduce) -> xs^T.
  Launch 2: weights are sharded over the hidden dim; core i owns
     W_v[:, 128i:128(i+1)] and w_o[128i:128(i+1), :] and computes its
     rank-128 contribution r_i = (xs @ Wv_i) @ wo_i  [4, 1024] via
     t = sum_a xsT_a.T @ wv_a  (stationary operand = tiny xsT tile),
     then r_i = transpose(t).T @ wo.  Both weight operands are
     pre-arranged on the host into the exact SBUF layout so every DMA is
     a contiguous 2D copy.
     Host sums the 8 partials -> r, and broadcasts r over the sequence
     dim to the full [4, 2048, 1024] output (the attention matrix is
     all-ones, so every sequence position carries the same row).
"""

import numpy as np

import concourse.bacc as bacc
import concourse.mybir as mybir
import concourse.tile as tile
from concourse.bass_utils import run_bass_kernel_spmd

NCORES = 8
B, N, D, H = 4, 2048, 1024, 1024
F32 = mybir.dt.float32
CHUNK = H // NCORES  # 128 hidden channels per core in launch 2
AX = mybir.AxisListType
ALU = mybir.AluOpType

_BUILT = {}


def _build_l1():
    """Row-reduce x-slice [1024, 1024] into poT [128, 8].

    poT[p, a] = sum over the slice's 1024 rows of x[:, 128a + p].
    """
    nc = bacc.Bacc("TRN2", target_bir_lowering=False, debug=False,
                   num_devices=NCORES)
    xs_ = nc.dram_tensor("xslice", [1024, 1024], F32, kind="ExternalInput")
    idm = nc.dram_tensor("idm", [128, 128], F32, kind="ExternalInput")
    pxT = nc.dram_tensor("pxT", [128, 8], F32, kind="ExternalOutput")

    with tile.TileContext(nc) as tc:
        with (
            tc.tile_pool(name="sbuf", bufs=8) as pool,
            tc.tile_pool(name="cst", bufs=1) as cst,
            tc.tile_pool(name="psum", bufs=2, space="PSUM") as psum,
            tc.tile_pool(name="warm", bufs=1, space="PSUM") as wpsum,
        ):
            idm_sb = cst.tile([128, 128], F32)
            nc.sync.dma_start(idm_sb[:], idm[:])
            xts = []
            for j in range(8):
                xt = pool.tile([128, 1024], F32)
                nc.sync.dma_start(xt[:], xs_[128 * j : 128 * (j + 1), :])
                xts.append(xt)

            # PE warm-up spread across the DMA window (HAM stays hot until
            # the transposes below).  Inputs are tiles that arrive early.
            wp = wpsum.tile([4, 512], F32)
            for _ in range(4):
                nc.tensor.matmul(wp[:], idm_sb[:, :4], xts[0][:, :512])
            for _ in range(3):
                nc.tensor.matmul(wp[:], idm_sb[:, :4], xts[4][:, :512])

            # running sum, paced by tile arrivals
            acc = cst.tile([128, 1024], F32)
            nc.vector.tensor_add(acc[:], xts[0][:], xts[1][:])
            for j in range(2, 8):
                nc.vector.tensor_add(acc[:], acc[:], xts[j][:])

            # partition fold: PE-transpose the 8 [128, 128] chunks into two
            # packed PSUM banks, then one batched DVE reduce per bank.
            po = cst.tile([128, 8], F32)
            for half in range(2):
                tp = psum.tile([128, 4, 128], F32)
                for u in range(4):
                    a = 4 * half + u
                    nc.tensor.transpose(tp[:, u, :],
                                        acc[:, 128 * a : 128 * (a + 1)],
                                        idm_sb[:])
                nc.vector.tensor_reduce(po[:, 4 * half : 4 * half + 4],
                                        tp[:], axis=AX.X, op=ALU.add)
            nc.sync.dma_start(pxT[:], po[:])
    nc.compile()
    return nc


def _build_l2():
    """r_part [4, 1024] = (xs @ Wv_chunk) @ wo_chunk for this core's chunk.

    xsT: [128, 32] host-packed so that column 4a+b is xs[b, 128a:128(a+1)].
    wv:  [128, 1024] host-packed so cols [128a:128(a+1)] are
         W_v[128a:128(a+1), chunk].
    wo:  [128, 1024] natural w_o[chunk, :].
    """
    nc = bacc.Bacc("TRN2", target_bir_lowering=False, debug=False,
                   num_devices=NCORES)
    xsT = nc.dram_tensor("xsT", [128, 32], F32, kind="ExternalInput")
    wv = nc.dram_tensor("wv", [128, 1024], F32, kind="ExternalInput")
    wo = nc.dram_tensor("wo", [128, 1024], F32, kind="ExternalInput")
    id4 = nc.dram_tensor("id4", [4, 4], F32, kind="ExternalInput")
    rp = nc.dram_tensor("rpart", [4, 1024], F32, kind="ExternalOutput")

    with tile.TileContext(nc) as tc:
        with (
            tc.tile_pool(name="sbuf", bufs=1) as pool,
            tc.tile_pool(name="psum", bufs=1, space="PSUM") as psum,
        ):
            xsT_sb = pool.tile([128, 32], F32)
            nc.sync.dma_start(xsT_sb[:], xsT[:])
            id4_sb = pool.tile([4, 4], F32)
            nc.sync.dma_start(id4_sb[:], id4[:])
            wv_sb = pool.tile([128, 1024], F32)
            nc.sync.dma_start(wv_sb[:], wv[:])
            wo_sb = pool.tile([128, 1024], F32)
            nc.scalar.dma_start(wo_sb[:], wo[:])

            # PE warm-up during the load window
            wp = psum.tile([4, 32], F32)
            for _ in range(14):
                nc.tensor.matmul(wp[:], xsT_sb[:, :4], xsT_sb[:])

            # t [4, 128] = sum_a xsT_a.T @ wv_a  (= xs @ Wv_chunk)
            pt = psum.tile([4, 128], F32)
            for a in range(8):
                nc.tensor.matmul(pt[:], xsT_sb[:, 4 * a : 4 * (a + 1)],
                                 wv_sb[:, 128 * a : 128 * (a + 1)],
                                 start=(a == 0), stop=(a == 7))
            t_sb = pool.tile([4, 128], F32)
            nc.vector.tensor_copy(t_sb[:], pt[:])

            # tT [128, 4] via PE transpose
            ptT = psum.tile([128, 4], F32)
            nc.tensor.transpose(ptT[:], t_sb[:], id4_sb[:])
            tT_sb = pool.tile([128, 4], F32)
            nc.vector.tensor_copy(tT_sb[:], ptT[:])

            # r_part [4, 1024] = tT.T @ wo_chunk (one 2-bank PSUM tile,
            # one batched copy out)
            pr = psum.tile([4, 1024], F32)
            nc.tensor.matmul(pr[:, :512], tT_sb[:], wo_sb[:, :512])
            nc.tensor.matmul(pr[:, 512:], tT_sb[:], wo_sb[:, 512:])
            ro = pool.tile([4, 1024], F32)
            nc.vector.tensor_copy(ro[:], pr[:])
            nc.sync.dma_start(rp[:], ro[:])
    nc.compile()
    return nc


def _get(name, builder):
    if name not in _BUILT:
        _BUILT[name] = builder()
    return _BUILT[name]


def kernel(x, w_qkv, w_o, alpha):
    x = np.ascontiguousarray(np.asarray(x, dtype=np.float32))
    w_qkv = np.asarray(w_qkv, dtype=np.float32)
    w_o = np.ascontiguousarray(np.asarray(w_o, dtype=np.float32))
    core_ids = list(range(NCORES))

    # ---- Launch 1: row-reduce x across all 8 cores -----------------------
    nc1 = _get("l1", _build_l1)
    xflat = x.reshape(B * N, D)  # rows [1024*i : 1024*(i+1)) belong to batch i//2
    idm = np.eye(128, dtype=np.float32)
    in_maps1 = [
        {"xslice": xflat[1024 * i : 1024 * (i + 1)], "idm": idm}
        for i in range(NCORES)
    ]
    res1 = run_bass_kernel_spmd(nc1, in_maps1, core_ids)
    # poT[p, a] -> px[128a + p]
    pxs = [r["pxT"].T.reshape(D) for r in res1.results]
    # cores 2b and 2b+1 each reduced one half of batch b
    xs = np.stack([pxs[2 * b] + pxs[2 * b + 1] for b in range(B)])  # [4, 1024]

    # ---- Launch 2: (xs @ Wv_chunk) @ wo_chunk, hidden dim sharded --------
    nc2 = _get("l2", _build_l2)
    # xsT packed [128, 32]: col 4a+b = xs[b, 128a:128(a+1)]
    xsT = np.ascontiguousarray(
        xs.reshape(B, 8, 128).transpose(2, 1, 0).reshape(128, 32)
    )
    id4 = np.eye(4, dtype=np.float32)
    in_maps2 = []
    for i in range(NCORES):
        c0, c1 = CHUNK * i, CHUNK * (i + 1)
        wv_chunk = w_qkv[:, 2 * H + c0 : 2 * H + c1]  # [1024, 128]
        wv_packed = np.ascontiguousarray(
            wv_chunk.reshape(8, 128, 128).transpose(1, 0, 2).reshape(128, 1024)
        )
        in_maps2.append({
            "xsT": xsT,
            "wv": wv_packed,
            "wo": np.ascontiguousarray(w_o[c0:c1, :]),
            "id4": id4,
        })
    res2 = run_bass_kernel_spmd(nc2, in_maps2, core_ids)
    r = np.sum([res["rpart"] for res in res2.results], axis=0)  # [4, 1024]

    # ---- Unshard: the score-normalized attention is all-ones, so every
    # sequence position of batch b carries the same row r[b].
    out = np.broadcast_to(r[:, None, :], (B, N, D))
    return np.ascontiguousarray(out)
